# revision 1
# baseline (speedup 1.0000x reference)
"""Trainium2 Bass kernel for per-pixel bucketed 3x3 conv (RAISR-style).

Problem: out[b,o,h,w] = sum_p patches[b,p,h,w] * W[buckets[b,h,w], o, p] + bias
  B=4, Cin=8, Cout=8, K=3, H=W=256, NUM_TYPES=216 filter buckets.

Strategy (8 NeuronCores, data-parallel over H with k//2 halo):
  - Each core owns 32 rows of H for all 4 batch images: 128 (b,h) pairs ->
    the 128 SBUF partitions; w (256) along the free axis.
  - Host prepares (data-independent layout transforms only):
      * im2col patches, bf16, [128, 256, 80] per core (72 features + ones
        row for bias + pad to 80 so the DVE reduce tree stays even/aligned)
      * the 216-row filter table re-laid to [256, 640] bf16 rows
        (per out-channel 80-element blocks: 72 weights + bias + pad)
      * bucket ids as int16 in the dma_gather "wrapped" index layout, plus
        bf16 bucket rows + iota columns for the PE-path one-hot compare
  - Device per core, 16 w-block groups of 2048 pixels each:
      * 12 groups fetch per-pixel weight rows via SWDGE dma_gather (HBM
        table -> SBUF, pixel -> partition). This path is bound by the Q7
        descriptor-generation rate (~9 ns/row), so
      * 4 groups instead compute their weight rows on TensorE: ScalarE
        builds a 216-way one-hot of the bucket ids (Relu(1 - Abs(b - t))
        with per-partition iota bias), two accumulating matmuls per
        w-column against the SBUF-resident table gather the rows into
        PSUM, and ScalarE drains PSUM -> SBUF (bf16 cast). These groups
        are interleaved among the gather groups in the DVE issue order.
      * All groups: DVE bf16 multiply (patches broadcast over the 8 output
        channels) then a 2x-mode binary add tree (80->40->20->10) and one
        final 1x reduce -> f32 [128, 8, 256]; w-range-chunked output DMAs.
  Measured: ~267-273 us on silicon (8 cores), rel err ~4e-3 vs the fp32
  reference (bf16 rounding).
"""

import numpy as np

B, Cin, Cout, K, H, W = 4, 8, 8, 3, 256, 256
NUM_TYPES = 216
NCORES = 8
RH = H // NCORES          # 32 rows of H per core
P = 128                   # partitions = B * RH
KPAD = 80                 # per-o feature block (72 weights + bias + pad to 80
                          # so the binary reduce tree stays even/aligned)
ROWLEN = 640              # table row length in bf16 elems = 8*80 exactly
PATLEN = 80               # patch row stride per pixel
NPX = B * RH * W          # pixels per core = 32768
GROUPS = 16               # DVE op groups per core
IDX_PER_CALL = NPX // GROUPS  # 2048 (needs single_packet=False: >64
                              # descriptors per DMA engine per call)
BLKS = W // GROUPS        # 16 w-columns per group
PE_GROUPS = 4             # trailing groups whose weight rows come from
                          # TensorE one-hot matmuls instead of the Q7 gather
                          # (balances the SWDGE descriptor-generation wall)

_COMPILED = {}


def _build_nc():
    from concourse import bacc, mybir
    from concourse.tile import TileContext

    nc = bacc.Bacc(None, target_bir_lowering=False, debug=False)
    bf16 = mybir.dt.bfloat16
    QG = GROUPS - PE_GROUPS  # gather-path groups
    NPECOL = PE_GROUPS * BLKS + 2 * (BLKS // 2)  # w-columns served by PE path
    # (trailing PE_GROUPS full groups + back halves of gather groups 10, 11)
    pat_ext = nc.declare_dram_parameter("pat", [P, W * PATLEN], bf16, isOutput=False)
    wtab_ext = nc.declare_dram_parameter("wtab", [256, ROWLEN], bf16, isOutput=False)
    bidx_ext = nc.declare_dram_parameter(
        "bidx", [P, QG * (IDX_PER_CALL // 16)], mybir.dt.int16, isOutput=False
    )
    bt_ext = nc.declare_dram_parameter("btf", [P, NPECOL * P], bf16, isOutput=False)
    iota_ext = nc.declare_dram_parameter("iotas", [P, 4], mybir.dt.float32, isOutput=False)
    out_ext = nc.declare_dram_parameter("out", [P, Cout * W], mybir.dt.float32, isOutput=True)

    from concourse import library_config

    with TileContext(nc) as tc:
        with (
            tc.tile_pool(name="main", bufs=1) as mpool,
            tc.tile_pool(name="wg", bufs=3) as wpool,
            tc.tile_pool(name="wgpe", bufs=1) as wpepool,
            tc.tile_pool(name="prod", bufs=1) as ppool,
            tc.tile_pool(name="tr", bufs=1) as trpool,
            tc.tile_pool(name="oh", bufs=1) as ohpool,
            tc.tile_pool(name="ps", bufs=4, space="PSUM") as pspool,
        ):
            nc.gpsimd.load_library(library_config.mlp)
            icols = IDX_PER_CALL // 16  # idx cols per gather call
            bidx_sb = mpool.tile([P, QG * icols], mybir.dt.int16)
            iota_sb = mpool.tile([P, 4], mybir.dt.float32)
            tab_sb = mpool.tile([P, 2 * ROWLEN], bf16)
            bt_sb = mpool.tile([P, NPECOL * P], bf16)
            # scalar-engine HWDGE queue: per-gather-call idx slices first so
            # the gathers are never starved, then the PE-path operands
            for c in range(QG):
                nc.scalar.dma_start(
                    out=bidx_sb[:, c * icols : (c + 1) * icols],
                    in_=bidx_ext[:, c * icols : (c + 1) * icols],
                )
            nc.scalar.dma_start(out=iota_sb[:], in_=iota_ext[:, :])
            nc.scalar.dma_start(
                out=tab_sb[:].rearrange("p (h f) -> p h f", f=ROWLEN),
                in_=wtab_ext[:, :].rearrange("(h p) f -> p h f", p=P),
            )
            nc.scalar.dma_start(out=bt_sb[:], in_=bt_ext[:, :])
            pat_sb = mpool.tile([P, W * PATLEN], bf16)
            qpat = W * PATLEN // 4
            for q in range(4):
                nc.sync.dma_start(
                    out=pat_sb[:, q * qpat : (q + 1) * qpat],
                    in_=pat_ext[:, q * qpat : (q + 1) * qpat],
                )
            out_sb = mpool.tile([P, Cout * W], mybir.dt.float32)

            pat3 = pat_sb[:].rearrange("p (w k) -> p w k", k=PATLEN)
            out3 = out_sb[:].rearrange("p (o w) -> p o w", w=W)
            tab3 = tab_sb[:].rearrange("p (h f) -> p h f", f=ROWLEN)

            def pe_fill(wg3, tstart, tcount, btbase):
                # one-hot on the (otherwise idle) ScalarE:
                # indicator(b == t) = Relu(1 - Abs(b - t)) for integer ids,
                # then 4 accumulating matmuls per w-column gather the weight
                # rows from the SBUF table into PSUM; ScalarE drains to SBUF
                oh = ohpool.tile([P, 2 * tcount * P], bf16, tag="oh")
                ohv = oh[:].rearrange("p (h t j) -> p h t j", h=2, j=P)
                ohflat = oh[:].rearrange("p (h f) -> p h f", h=2)
                ab = ohpool.tile([P, tcount * P], bf16, tag="ab")
                for h in range(2):
                    nc.scalar.activation(
                        out=ab[:, : tcount * P],
                        in_=bt_sb[:, btbase * P : (btbase + tcount) * P],
                        func=mybir.ActivationFunctionType.Abs,
                        bias=iota_sb[:, 2 + h : 3 + h],
                        scale=1.0,
                    )
                    nc.scalar.activation(
                        out=ohflat[:, h, :],
                        in_=ab[:, : tcount * P],
                        func=mybir.ActivationFunctionType.Relu,
                        bias=1.0,
                        scale=-1.0,
                    )
                for t in range(tcount):
                    psa = pspool.tile([P, 320], mybir.dt.float32, tag="psa")
                    psb = pspool.tile([P, 320], mybir.dt.float32, tag="psb")
                    for h in range(2):
                        nc.tensor.matmul(
                            out=psa[:],
                            lhsT=ohv[:, h, t, :],
                            rhs=tab3[:, h, :320],
                            start=(h == 0),
                            stop=(h == 1),
                        )
                        nc.tensor.matmul(
                            out=psb[:],
                            lhsT=ohv[:, h, t, :],
                            rhs=tab3[:, h, 320:],
                            start=(h == 0),
                            stop=(h == 1),
                        )
                    nc.scalar.activation(
                        out=wg3[:, tstart + t, :320],
                        in_=psa[:],
                        func=mybir.ActivationFunctionType.Copy,
                    )
                    nc.scalar.activation(
                        out=wg3[:, tstart + t, 320:],
                        in_=psb[:],
                        func=mybir.ActivationFunctionType.Copy,
                    )

            done: set[int] = set()
            # Interleave PE-path groups among gather groups so the DVE queue
            # alternates: each PE group's ~21us DVE burst fits in the slack
            # the (faster-than-gather) DVE accumulates over ~4 gather groups.
            order = [QG, 0, 1, 2, QG + 1, 3, 4, 5, QG + 2, 6, 7, 8, QG + 3, 9, 10, 11]
            assert sorted(order) == list(range(GROUPS))
            for c in order:
                if c < QG:
                    wg = wpool.tile([P, BLKS * ROWLEN], bf16, tag="wg")
                    wg3 = wg[:].rearrange("p (t f) -> p t f", f=ROWLEN)
                    half = c >= QG - 2  # last 2 gather groups: back half via PE
                    nidx = IDX_PER_CALL // 2 if half else IDX_PER_CALL
                    nc.gpsimd.dma_gather(
                        out_ap=wg3[:, : BLKS // 2, :] if half else wg3,
                        in_ap=wtab_ext[:216, :],
                        idxs_ap=bidx_sb[:, c * icols : c * icols + nidx // 16],
                        num_idxs=nidx,
                        num_idxs_reg=nidx,
                        elem_size=ROWLEN,
                        single_packet=False,
                    )
                    if half:
                        base = PE_GROUPS * BLKS + (QG - 1 - c) * (BLKS // 2)
                        pe_fill(wg3, BLKS // 2, BLKS // 2, base)
                else:
                    # PE path: per w-column one-hot matmuls against the
                    # SBUF-resident table; ScalarE drains PSUM -> wg tile
                    wg = wpepool.tile([P, BLKS * ROWLEN], bf16, tag="wgpe")
                    wg3 = wg[:].rearrange("p (t f) -> p t f", f=ROWLEN)
                    pe_fill(wg3, 0, BLKS, (c - QG) * BLKS)
                prod = ppool.tile([P, BLKS * Cout * KPAD], bf16, tag="prod")
                prod4 = prod[:].rearrange("p (t o k) -> p t o k", o=Cout, k=KPAD)
                pat_b = (
                    pat3[:, c * BLKS : (c + 1) * BLKS, :KPAD]
                    .unsqueeze(2)
                    .broadcast_to([P, BLKS, Cout, KPAD])
                )
                wg4 = wg[:].rearrange("p (t o k) -> p t o k", o=Cout, k=KPAD)
                nc.vector.tensor_tensor(
                    out=prod4, in0=pat_b, in1=wg4, op=mybir.AluOpType.mult
                )
                # binary-tree partial reduction at DVE 2x (bf16 tensor_tensor)
                # 80 -> 40 -> 20, then one 1x flat reduce over 20
                tr1 = trpool.tile([P, BLKS * Cout * 40], bf16, tag="tr1")
                t1v = tr1[:].rearrange("p (t o k) -> p t o k", o=Cout, k=40)
                nc.vector.tensor_tensor(
                    out=t1v,
                    in0=prod4[:, :, :, :40],
                    in1=prod4[:, :, :, 40:],
                    op=mybir.AluOpType.add,
                )
                tr2 = trpool.tile([P, BLKS * Cout * 20], bf16, tag="tr2")
                t2v = tr2[:].rearrange("p (t o k) -> p t o k", o=Cout, k=20)
                nc.vector.tensor_tensor(
                    out=t2v,
                    in0=t1v[:, :, :, :20],
                    in1=t1v[:, :, :, 20:],
                    op=mybir.AluOpType.add,
                )
                tr3 = trpool.tile([P, BLKS * Cout * 10], bf16, tag="tr3")
                t3v = tr3[:].rearrange("p (t o k) -> p t o k", o=Cout, k=10)
                nc.vector.tensor_tensor(
                    out=t3v,
                    in0=t2v[:, :, :, :10],
                    in1=t2v[:, :, :, 10:],
                    op=mybir.AluOpType.add,
                )
                nc.vector.tensor_reduce(
                    out=out3[:, :, c * BLKS : (c + 1) * BLKS].transpose([0, 2, 1]),
                    in_=t3v,
                    axis=mybir.AxisListType.X,
                    op=mybir.AluOpType.add,
                )

                done.add(c)
                for q in range(4):  # drain fully-finished w-ranges to HBM
                    qs = set(range(q * 4, q * 4 + 4))
                    if qs <= done and not (qs <= (done - {c})):
                        oext3 = out_ext[:, :].rearrange("p (o w) -> p o w", w=W)
                        nc.sync.dma_start(
                            out=oext3[:, :, q * 64 : (q + 1) * 64],
                            in_=out3[:, :, q * 64 : (q + 1) * 64],
                        )
    nc.compile()
    return nc


def _prep_inputs(x, filter_emb, buckets):
    """Host-side data-independent layout prep. Returns in_maps for 8 cores."""
    import ml_dtypes

    bf16 = ml_dtypes.bfloat16
    x = np.asarray(x, dtype=np.float32)
    filter_emb = np.asarray(filter_emb, dtype=np.float32)
    buckets = np.asarray(buckets).astype(np.int64)

    # --- weight table: [256, 640] bf16 (216 used), row = per-o 80-blocks ---
    nw = Cout * Cin * K * K
    wtab = np.zeros((256, ROWLEN), dtype=np.float32)
    wmat = filter_emb[:, :nw].reshape(NUM_TYPES, Cout, Cin * K * K)
    bias = filter_emb[:, nw:]  # [216, 8]
    for o in range(Cout):
        wtab[:NUM_TYPES, o * KPAD : o * KPAD + 72] = wmat[:, o, :]
        wtab[:NUM_TYPES, o * KPAD + 72] = bias[:, o]
    wtab = wtab.astype(bf16)
    ar = np.arange(P, dtype=np.float32)
    iotas = np.stack([ar, ar + P, -ar, -(ar + P)], axis=1)
    QG = GROUPS - PE_GROUPS

    # --- im2col patches, feature order (c, kh, kw) ---
    xp = np.pad(x, ((0, 0), (0, 0), (1, 1), (1, 1)))
    sw = np.lib.stride_tricks.sliding_window_view(xp, (K, K), axis=(2, 3))
    # sw: [B, Cin, H, W, K, K] -> [B, H, W, Cin*K*K]
    patches = sw.transpose(0, 2, 3, 1, 4, 5).reshape(B, H, W, Cin * K * K)

    in_maps = []
    for ci in range(NCORES):
        h0 = ci * RH
        # pat [128=(b,hl), W, 80]
        pat = np.zeros((P, W, PATLEN), dtype=np.float32)
        pslab = patches[:, h0 : h0 + RH]  # [B, RH, W, 72]
        pat[:, :, :72] = pslab.reshape(P, W, 72)
        pat[:, :, 72] = 1.0
        pat = pat.astype(bf16).reshape(P, W * PATLEN)

        # bucket ids in dma_gather wrapped layout
        bcore = buckets[:, h0 : h0 + RH].reshape(P, W).astype(np.int16)
        # gather call c covers w in [c*BLKS, (c+1)*BLKS); position i in the
        # call -> pixel (part=i%128, w = c*BLKS + i//128); idx position i
        # lives at [partition i%16, col i//16], replicated across the 8
        # 16-partition groups
        icols = IDX_PER_CALL // 16
        bidx = np.zeros((P, QG, icols), dtype=np.int16)
        pmat = np.arange(P)[:, None] % 16  # [P,1]
        imat = np.arange(icols)[None, :] * 16 + pmat  # [P, icols] position i
        for c in range(QG):
            part = imat % P
            wcol = c * BLKS + imat // P
            if c >= QG - 2:
                # last two gather groups: only the front half of the w-range
                # is gathered (the back halves come from the PE path)
                sub = imat[:, : icols // 2]
                bidx[:, c, : icols // 2] = bcore[sub % P, c * BLKS + sub // P]
            else:
                bidx[:, c, :] = bcore[part, wcol]
        bidx = bidx.reshape(P, QG * icols)

        # PE-path bucket rows, replicated down all 128 partitions:
        # cols 0..PE_GROUPS*BLKS-1 -> w = QG*BLKS..255 (full PE groups),
        # then BLKS//2 cols for the back half of gather group QG-1
        hb = BLKS // 2
        pecols = np.concatenate(
            [
                bcore[:, QG * BLKS :],                              # full PE groups
                bcore[:, (QG - 1) * BLKS + hb : QG * BLKS],         # group QG-1 back
                bcore[:, (QG - 2) * BLKS + hb : (QG - 1) * BLKS],   # group QG-2 back
            ],
            axis=1,
        )
        NPECOL = PE_GROUPS * BLKS + 2 * hb
        btf = np.tile(
            pecols.astype(np.float32).T.reshape(1, NPECOL * P), (P, 1)
        ).astype(bf16)

        in_maps.append(
            {"pat": pat, "wtab": wtab, "bidx": bidx, "btf": btf, "iotas": iotas}
        )
    return in_maps


def kernel(x, filter_emb, buckets):
    from concourse.bass_utils import run_bass_kernel_spmd

    if "nc" not in _COMPILED:
        _COMPILED["nc"] = _build_nc()
    nc = _COMPILED["nc"]

    in_maps = _prep_inputs(x, filter_emb, buckets)
    res = run_bass_kernel_spmd(nc, in_maps, core_ids=list(range(NCORES)))

    out = np.empty((B, Cout, H, W), dtype=np.float32)
    for ci in range(NCORES):
        o = np.asarray(res.results[ci]["out"], dtype=np.float32).reshape(P, Cout, W)
        # partition p = (b = p//RH, hl = p%RH)
        out[:, :, ci * RH : (ci + 1) * RH, :] = o.reshape(B, RH, Cout, W).transpose(
            0, 2, 1, 3
        )
    return out

